# revision 5
# baseline (speedup 1.0000x reference)
"""Trainium2 Bass kernel for BilinearDecoder GNN edge scoring.

scores[e] = sum_j (z[src_e] @ W)[j] * z[dst_e][j] + bias

Strategy (pure data-parallel over edges, 8 cores):
  - z [100000, 64] f32 and W [64, 64] replicated to every core.
  - edge_index sharded along edges: 125000 edges/core, padded to 131072.
  - Per core: 64 chunks of 2048 edges. For each chunk
      * gather z[src] and z[dst] rows (256 B each) from HBM via
        indirect DMA: one offset per partition, 128 rows per
        instruction (the only indirect-DMA shape this platform's
        compiler lowers correctly),
      * per 128-column group (2 edge-slots x 64 dims): PE-transpose
        the src rows, multiply by a block-diagonal [[W,0],[0,W]] so one
        matmul applies W to 2 slots at once, then elementwise-multiply
        with the dst rows and segment-reduce on DVE.
  - Scores DMA'd back as contiguous [2048] per chunk; host concatenates
    the 8 padded shards and trims to 1M edges.
"""

import numpy as np

import concourse.bass as bass
import concourse.mybir as mybir
from concourse import bacc
from concourse.bass import IndirectOffsetOnAxis
from concourse.bass_utils import run_bass_kernel_spmd
from concourse.masks import make_identity
from concourse.tile import TileContext

N_CORES = 8
N_NODES = 100000
DIM = 64
N_EDGES = 1000000
E_PER_CORE = N_EDGES // N_CORES  # 125000
K_SLOTS = 16                     # gathered rows per partition per chunk
CHUNK = 128 * K_SLOTS            # 2048 edges per chunk
N_CHUNKS = -(-E_PER_CORE // CHUNK)  # 62 -> pad
E_PAD = CHUNK * N_CHUNKS

F32 = mybir.dt.float32
I32 = mybir.dt.int32


def build_bass(n_nodes=N_NODES, n_chunks=N_CHUNKS):
    e_pad = n_chunks * CHUNK
    nc = bacc.Bacc()
    z_d = nc.declare_dram_parameter("z", [n_nodes, DIM], F32, isOutput=False)
    w_d = nc.declare_dram_parameter("W", [DIM, DIM], F32, isOutput=False)
    bias_d = nc.declare_dram_parameter("biasb", [128, 1], F32, isOutput=False)
    src_d = nc.declare_dram_parameter("src", [e_pad], I32, isOutput=False)
    dst_d = nc.declare_dram_parameter("dst", [e_pad], I32, isOutput=False)
    out_d = nc.declare_dram_parameter("out", [e_pad], F32, isOutput=True)

    with TileContext(nc) as tc:
        with (
            tc.tile_pool(name="const", bufs=1) as cpool,
            tc.tile_pool(name="gather", bufs=3) as gpool,
            tc.tile_pool(name="work", bufs=3) as wpool,
            tc.tile_pool(name="ps", bufs=3, space="PSUM") as ppool,
        ):
            ident = cpool.tile([128, 128], F32)
            make_identity(nc, ident[:])
            # Block-diagonal [[W, 0], [0, W]] so one 128x128 matmul applies W
            # to two 64-wide slots at once.
            wbd = cpool.tile([128, 128], F32)
            nc.vector.memset(wbd[:], 0.0)
            nc.sync.dma_start(out=wbd[0:64, 0:64], in_=w_d[:, :])
            nc.sync.dma_start(out=wbd[64:128, 64:128], in_=w_d[:, :])
            bias_t = cpool.tile([128, 1], F32)
            nc.sync.dma_start(out=bias_t[:], in_=bias_d[:, :])

            for c in range(n_chunks):
                sl = slice(c * CHUNK, (c + 1) * CHUNK)
                idx_s = gpool.tile([128, K_SLOTS], I32, tag="idxs")
                nc.sync.dma_start(
                    out=idx_s[:], in_=src_d[sl].rearrange("(p k) -> p k", p=128)
                )
                idx_t = gpool.tile([128, K_SLOTS], I32, tag="idxd")
                nc.sync.dma_start(
                    out=idx_t[:], in_=dst_d[sl].rearrange("(p k) -> p k", p=128)
                )
                # a_t[p, k*64+d] = z[src[chunk, p, k], d]; same for b_t/dst.
                # One indirect DMA per slot column: 128 offsets (one per
                # partition), 256 B per descriptor.
                a_t = gpool.tile([128, K_SLOTS * DIM], F32, tag="A")
                b_t = gpool.tile([128, K_SLOTS * DIM], F32, tag="B")
                for j in range(K_SLOTS):
                    nc.gpsimd.indirect_dma_start(
                        out=a_t[:, j * DIM:(j + 1) * DIM],
                        out_offset=None,
                        in_=z_d[:],
                        in_offset=IndirectOffsetOnAxis(
                            ap=idx_s[:, j:j + 1], axis=0
                        ),
                    )
                for j in range(K_SLOTS):
                    nc.gpsimd.indirect_dma_start(
                        out=b_t[:, j * DIM:(j + 1) * DIM],
                        out_offset=None,
                        in_=z_d[:],
                        in_offset=IndirectOffsetOnAxis(
                            ap=idx_t[:, j:j + 1], axis=0
                        ),
                    )
                scores = wpool.tile([128, K_SLOTS], F32, tag="scores")
                for g in range(K_SLOTS // 2):
                    fs = slice(g * 128, (g + 1) * 128)
                    tp = ppool.tile([128, 128], F32, tag="tp")
                    nc.tensor.transpose(out=tp[:], in_=a_t[:, fs], identity=ident[:])
                    at = wpool.tile([128, 128], F32, tag="at")
                    nc.scalar.copy(out=at[:], in_=tp[:])
                    cp = ppool.tile([128, 128], F32, tag="cp")
                    nc.tensor.matmul(
                        out=cp[:], lhsT=at[:], rhs=wbd[:], start=True, stop=True
                    )
                    prod = wpool.tile([128, 128], F32, tag="prod")
                    nc.vector.tensor_tensor(
                        out=prod[:], in0=cp[:], in1=b_t[:, fs],
                        op=mybir.AluOpType.mult,
                    )
                    nc.vector.reduce_sum(
                        out=scores[:, g * 2:(g + 1) * 2],
                        in_=prod[:].rearrange("p (s d) -> p s d", d=DIM),
                        axis=mybir.AxisListType.X,
                    )
                nc.vector.tensor_scalar_add(
                    out=scores[:], in0=scores[:], scalar1=bias_t[:, :1]
                )
                nc.sync.dma_start(
                    out=out_d[sl].rearrange("(p k) -> p k", p=128), in_=scores[:]
                )
    nc.compile()
    return nc


_CACHE = {}


def _get_nc():
    if "nc" not in _CACHE:
        _CACHE["nc"] = build_bass()
    return _CACHE["nc"]


def _make_in_maps(z, edge_index, W, bias):
    z = np.ascontiguousarray(np.asarray(z, dtype=np.float32))
    W = np.ascontiguousarray(np.asarray(W, dtype=np.float32))
    bias_f = np.float32(np.asarray(bias).reshape(-1)[0])
    ei = np.asarray(edge_index)
    src = np.ascontiguousarray(ei[0].astype(np.int32))
    dst = np.ascontiguousarray(ei[1].astype(np.int32))
    biasb = np.full((128, 1), bias_f, dtype=np.float32)
    in_maps = []
    for c in range(N_CORES):
        lo, hi = c * E_PER_CORE, (c + 1) * E_PER_CORE
        s = np.zeros(E_PAD, np.int32)
        s[:E_PER_CORE] = src[lo:hi]
        d = np.zeros(E_PAD, np.int32)
        d[:E_PER_CORE] = dst[lo:hi]
        in_maps.append({"z": z, "W": W, "biasb": biasb, "src": s, "dst": d})
    return in_maps


def _gather_out(res):
    parts = [
        np.asarray(res.results[c]["out"]).reshape(-1)[:E_PER_CORE]
        for c in range(N_CORES)
    ]
    return np.concatenate(parts)


def kernel(z, edge_index, W, bias):
    in_maps = _make_in_maps(z, edge_index, W, bias)
    res = run_bass_kernel_spmd(_get_nc(), in_maps, list(range(N_CORES)))
    return _gather_out(res)


def kernel_traced(z, edge_index, W, bias):
    """Like kernel() but also returns the profiled HW exec time in ns."""
    in_maps = _make_in_maps(z, edge_index, W, bias)
    res = run_bass_kernel_spmd(
        _get_nc(), in_maps, list(range(N_CORES)), trace=True
    )
    return _gather_out(res), res.exec_time_ns
